# revision 4
# baseline (speedup 1.0000x reference)
"""CMCD sampler kernel for Trainium2 (8 NeuronCores, data-parallel over batch).

Problem: nn_CMCD_84877143704251.

reference semantics (B=8192, ZD=256, CD=256, N=32, H=512):
  z0, eps ~ fixed jax RNG (key 42)
  per step i:
    mu_f  = z + (sig_f^2 * grad(z, beta_f) + ctrl(tf, z)) * dt
    z'    = mu_f + sig_f*sqrt(dt)*eps_i
    mu_b  = z' + (sig_b^2 * grad(z', beta_b) - ctrl(tb, z')) * dt
    log_w += lpdf(z, mu_b, s_b) - lpdf(z', mu_f, s_f)
  grad(z, beta) = (1-beta)(-z) + beta*(-(z-mu_T)/sigma_T^2)
  ctrl(t, z) = tanh([z, ctx, t] @ W1 + b1) @ W2 + b2
  outputs: (log_w_scalar, z_chain (N+1, B, ZD))

Device strategy (per core, batch shard BL=1024, feature-major layout
[feature partitions, batch free]):
  - sigma_T == 1 (as produced by setup_inputs) lets grad simplify to
    beta*mu_T - z. Verified at runtime; otherwise fall back to numpy.
  - z' = af_i * z + dt*(tanh(FL + bias_f) @ W2) + e'_i  with
      FL    = z @ W1[:ZD] + ctx @ W1[ZD:ZD+CD]        (PE, ctx part cached)
      bias  = b1 + t * W1[-1]                          (per-partition ACT bias)
      e'_i  = s_f*eps_i + dt*sig_f^2*beta_f*mu_T + dt*b2   (host precomputed)
  - d_i = (z - mu_b) - dt*b2
        = zprev - bh*z' - ch*mu_T + dt*(tanh(FL' + bias_b) @ W2)
    Only per-(partition,step) mean/var of d are shipped out (bn_stats);
    host reconstructs sum((d + dt*b2)^2)/s_b^2.
  - log_w assembled on host: forward quadratic == sum(0.5*eps^2), per-step
    log-sigma constants cancel over the full loop (permutation), endpoint
    terms computed from z_chain on host.
"""

import math
from functools import lru_cache

import numpy as np

B, ZD, CD, NSTEPS, H = 8192, 256, 256, 32, 512
NCORES = 8
BL = B // NCORES           # batch per core
BC = 512                   # batch chunk processed per psum tile
NBC = BL // BC             # 2
NZT = ZD // 128            # zd partition tiles
NCT = CD // 128            # ctx partition tiles
NHT = H // 128             # h partition tiles
DT = 1.0 / NSTEPS

_PROG_CACHE: dict = {}


def _schedule_constants(sigma_sched, beta_sched):
    sf = np.asarray(sigma_sched, np.float64)
    bf = np.asarray(beta_sched, np.float64)
    idx_b = (np.arange(NSTEPS) - 1) % NSTEPS
    sb = sf[idx_b]
    bb = bf[idx_b]
    return {
        "af": 1.0 - DT * sf**2,
        "cf": DT * sf**2 * bf,
        "s_f": sf * math.sqrt(DT),
        "bh": 1.0 - DT * sb**2,
        "ch": DT * sb**2 * bb,
        "s_b": sb * math.sqrt(DT),
    }


def _build_program(af, bh, ch):
    """Emit the per-core Bass program. af/bh/ch are tuples of python floats
    (baked as immediates)."""
    from contextlib import ExitStack

    import concourse.tile as tile
    from concourse import bacc, mybir

    f32 = mybir.dt.float32
    f32r = mybir.dt.float32r
    MULT = mybir.AluOpType.mult
    ADD = mybir.AluOpType.add
    TANH = mybir.ActivationFunctionType.Tanh

    nc = bacc.Bacc("TRN2", target_bir_lowering=False, debug=False,
                   num_devices=NCORES)

    def inp(name, shape, dt=f32):
        return nc.dram_tensor(name, list(shape), dt, kind="ExternalInput").ap()

    def outp(name, shape, dt=f32):
        return nc.dram_tensor(name, list(shape), dt, kind="ExternalOutput").ap()

    z0_d = inp("z0", (ZD, BL), f32r)
    ctx_d = inp("ctx", (CD, BL), f32r)
    mut_d = inp("mut", (ZD, BL), f32r)
    eps_d = inp("eps", (NSTEPS, ZD, BL), f32r)
    w1z_d = inp("w1z", (128, NZT, NHT, 128), f32r)
    w1c_d = inp("w1c", (128, NCT, NHT, 128), f32r)
    w2_d = inp("w2", (128, NHT, NZT, 128), f32r)
    idm_d = inp("idm", (128, 128), f32r)
    bia_d = inp("bia", (128, NHT, 2, NSTEPS))
    zs_d = outp("zs", (NSTEPS, ZD, BL), f32r)
    sts_d = outp("sts", (128, NZT, NBC, NSTEPS, 2))

    with ExitStack() as ctxs:
        tc = ctxs.enter_context(tile.TileContext(nc))
        const = ctxs.enter_context(tc.tile_pool(name="const", bufs=1))
        zpool = ctxs.enter_context(tc.tile_pool(name="zpool", bufs=3))
        epool = ctxs.enter_context(tc.tile_pool(name="epool", bufs=3))
        hpool = ctxs.enter_context(tc.tile_pool(name="hpool", bufs=4))
        dpool = ctxs.enter_context(tc.tile_pool(name="dpool", bufs=4))
        spool = ctxs.enter_context(tc.tile_pool(name="spool", bufs=4))
        icpool = ctxs.enter_context(tc.tile_pool(name="icpool", bufs=2))
        flp = ctxs.enter_context(tc.tile_pool(name="flp", bufs=1, space="PSUM"))
        psp = ctxs.enter_context(tc.tile_pool(name="psp", bufs=4, space="PSUM"))

        # ---- constants into SBUF ----
        w1z = const.tile([128, NZT, NHT, 128], f32r, name="w1z_sb")
        nc.sync.dma_start(out=w1z[:], in_=w1z_d)
        w1c = const.tile([128, NCT, NHT, 128], f32r, name="w1c_sb")
        nc.sync.dma_start(out=w1c[:], in_=w1c_d)
        w2 = const.tile([128, NHT, NZT, 128], f32r, name="w2_sb")
        nc.sync.dma_start(out=w2[:], in_=w2_d)
        idm = const.tile([128, 128], f32r, name="idm_sb")
        nc.sync.dma_start(out=idm[:], in_=idm_d)
        bia = const.tile([128, NHT, 2, NSTEPS], f32, name="bia_sb")
        nc.sync.dma_start(out=bia[:], in_=bia_d)
        mut = const.tile([128, NZT, BL], f32r, name="mut_sb")
        for j in range(NZT):
            nc.sync.dma_start(out=mut[:, j, :], in_=mut_d[128 * j:128 * (j + 1), :])
        ctxt = const.tile([128, NCT, BL], f32r, name="ctx_sb")
        for j in range(NCT):
            nc.sync.dma_start(out=ctxt[:, j, :], in_=ctx_d[128 * j:128 * (j + 1), :])
        stat = const.tile([128, NZT, NBC, NSTEPS, 2], f32, name="stat_sb")

        # ---- ctx part of first layer (constant over steps) ----
        ctxp = const.tile([128, NHT, BL], f32r, name="ctxp_sb")
        for jh in range(NHT):
            for c in range(NBC):
                sl = slice(c * BC, (c + 1) * BC)
                ps = psp.tile([128, BC], f32, name=f"ctxp_ps_{jh}_{c}", tag="ps")
                for kk in range(NCT):
                    nc.tensor.matmul(ps[:], w1c[:, kk, jh, :], ctxt[:, kk, sl],
                                     start=(kk == 0), stop=(kk == NCT - 1))
                nc.scalar.copy(out=ctxp[:, jh, sl], in_=ps[:])

        # ---- z0 ----
        zcur = zpool.tile([128, NZT, BL], f32r, name="z_0", tag="z")
        for j in range(NZT):
            nc.sync.dma_start(out=zcur[:, j, :], in_=z0_d[128 * j:128 * (j + 1), :])

        def load_eps(k):
            t = epool.tile([128, NZT, BL], f32r, name=f"eps_{k}", tag="eps")
            for j in range(NZT):
                nc.sync.dma_start(out=t[:, j, :],
                                  in_=eps_d[k, 128 * j:128 * (j + 1), :])
            return t

        eps_sb = {0: load_eps(0)}
        zprev = None

        # ---- main chain: k indexes the z being fed through the first layer ----
        for k in range(NSTEPS + 1):
            if k + 1 < NSTEPS:
                eps_sb[k + 1] = load_eps(k + 1)
            fwd = k < NSTEPS   # produce z_{k+1}
            bwd = k >= 1       # score step k-1 backward kernel

            if bwd:
                idc = icpool.tile([128, 128], f32r, name=f"idc_{k}", tag="idc")
                nc.vector.tensor_scalar_mul(idc[:], idm[:], float(-ch[k - 1]))

            znext = None
            if fwd:
                znext = zpool.tile([128, NZT, BL], f32r, name=f"z_{k + 1}", tag="z")

            for c in range(NBC):
                sl = slice(c * BC, (c + 1) * BC)
                # first layer for z_k (shared by bwd step k-1 and fwd step k)
                fl = flp.tile([128, NHT, BC], f32, name=f"fl_{k}_{c}", tag="fl")
                for jh in range(NHT):
                    for kk in range(NZT):
                        nc.tensor.matmul(fl[:, jh, :], w1z[:, kk, jh, :],
                                         zcur[:, kk, sl],
                                         start=(kk == 0), stop=False)
                    nc.tensor.matmul(fl[:, jh, :], idm[:], ctxp[:, jh, sl],
                                     start=False, stop=True)

                if bwd:
                    hb = hpool.tile([128, NHT, BC], f32r, name=f"hb_{k}_{c}", tag="h")
                    for jh in range(NHT):
                        nc.scalar.activation(hb[:, jh, :], fl[:, jh, :], TANH,
                                             bias=bia[:, jh, 0, k - 1:k], scale=1.0)
                hf = None
                if fwd:
                    hf = hpool.tile([128, NHT, BC], f32r, name=f"hf_{k}_{c}", tag="h")
                    for jh in range(NHT):
                        nc.scalar.activation(hf[:, jh, :], fl[:, jh, :], TANH,
                                             bias=bia[:, jh, 1, k:k + 1], scale=1.0)

                if bwd:
                    # d = zprev - bh*z_k - ch*mu_T + dt*(hb @ W2)
                    for j in range(NZT):
                        dps = psp.tile([128, BC], f32, name=f"dps_{k}_{c}_{j}",
                                       tag="ps")
                        for kk in range(NHT):
                            nc.tensor.matmul(dps[:], w2[:, kk, j, :],
                                             hb[:, kk, :],
                                             start=(kk == 0), stop=False)
                        nc.tensor.matmul(dps[:], idm[:], zprev[:, j, sl],
                                         start=False, stop=False)
                        nc.tensor.matmul(dps[:], idc[:], mut[:, j, sl],
                                         start=False, stop=True)
                        dsb = dpool.tile([128, BC], f32, name=f"d_{k}_{c}_{j}",
                                         tag="d")
                        nc.vector.scalar_tensor_tensor(
                            dsb[:], zcur[:, j, sl], float(-bh[k - 1]), dps[:],
                            MULT, ADD)
                        st6 = spool.tile([128, 6], f32, name=f"st6_{k}_{c}_{j}",
                                         tag="st6")
                        nc.vector.bn_stats(st6[:], dsb[:])
                        nc.vector.bn_aggr(stat[:, j, c, k - 1, :], st6[:])

                if fwd:
                    # z_{k+1} = af*z_k + dt*(hf @ W2) + e'_k
                    for j in range(NZT):
                        zps = psp.tile([128, BC], f32, name=f"zps_{k}_{c}_{j}",
                                       tag="ps")
                        for kk in range(NHT):
                            nc.tensor.matmul(zps[:], w2[:, kk, j, :],
                                             hf[:, kk, :],
                                             start=(kk == 0), stop=False)
                        nc.tensor.matmul(zps[:], idm[:], eps_sb[k][:, j, sl],
                                         start=False, stop=True)
                        nc.vector.scalar_tensor_tensor(
                            znext[:, j, sl], zcur[:, j, sl], float(af[k]), zps[:],
                            MULT, ADD)
                        nc.sync.dma_start(out=zs_d[k, 128 * j:128 * (j + 1), sl],
                                          in_=znext[:, j, sl])

            if fwd:
                zprev = zcur
                zcur = znext
            if k - 1 in eps_sb:
                del eps_sb[k - 1]

        nc.sync.dma_start(out=sts_d, in_=stat[:])

    nc.compile()
    return nc


def _get_program(consts):
    key = (tuple(consts["af"]), tuple(consts["bh"]), tuple(consts["ch"]))
    if key not in _PROG_CACHE:
        _PROG_CACHE.clear()
        _PROG_CACHE[key] = _build_program(consts["af"], consts["bh"], consts["ch"])
    return _PROG_CACHE[key]


_RNG_SCRIPT = """
import sys
import jax
import numpy as np
key = jax.random.key(42)
k0, k1 = jax.random.split(key)
z0 = np.asarray(jax.random.normal(k0, ({B}, {ZD}), np.float32))
eps = np.asarray(jax.random.normal(k1, ({N}, {B}, {ZD}), np.float32))
np.save(sys.argv[1], z0)
np.save(sys.argv[2], eps)
"""


@lru_cache(maxsize=1)
def _rng_draws():
    """Draw z0/eps exactly as the reference does. The reference runs under
    JAX_PLATFORMS=cpu and this container pins jax_default_prng_impl=rbg,
    whose bitstream is backend-dependent — so the draw must be lowered for
    the CPU backend. Done in a subprocess to be independent of this
    process's jax platform state."""
    import os
    import subprocess
    import sys
    import tempfile

    with tempfile.TemporaryDirectory() as td:
        pz = os.path.join(td, "z0.npy")
        pe = os.path.join(td, "eps.npy")
        env = dict(os.environ, JAX_PLATFORMS="cpu")
        script = _RNG_SCRIPT.format(B=B, ZD=ZD, N=NSTEPS)
        subprocess.run([sys.executable, "-c", script, pz, pe], env=env,
                       check=True, capture_output=True)
        z0 = np.load(pz)
        eps = np.load(pe)
    return z0, eps


def _host_inputs(context_embedding, mu_T, W1, b1, W2, b2, consts):
    z0, eps = _rng_draws()
    s_f = consts["s_f"].astype(np.float32)
    cf = consts["cf"].astype(np.float32)

    mut_t = np.ascontiguousarray(mu_T.T)                    # (ZD, B)
    ctx_t = np.ascontiguousarray(context_embedding.T)       # (CD, B)
    z0_t = np.ascontiguousarray(z0.T)                       # (ZD, B)
    # e' = s_f*eps + dt*sig_f^2*beta_f*mu_T + dt*b2  (feature-major)
    eps_t = np.ascontiguousarray(eps.transpose(0, 2, 1))    # (N, ZD, B)
    epr = (s_f[:, None, None] * eps_t
           + cf[:, None, None] * mut_t[None]
           + (DT * b2).astype(np.float32)[None, :, None])

    w1z = np.ascontiguousarray(
        W1[:ZD].reshape(NZT, 128, NHT, 128).transpose(1, 0, 2, 3))
    w1c = np.ascontiguousarray(
        W1[ZD:ZD + CD].reshape(NCT, 128, NHT, 128).transpose(1, 0, 2, 3))
    w2s = np.ascontiguousarray(
        (DT * W2.astype(np.float64)).astype(np.float32)
        .reshape(NHT, 128, NZT, 128).transpose(1, 0, 2, 3))
    idm = np.eye(128, dtype=np.float32)

    # bias[p, jh, kind, i] = b1[jh*128+p] + t * W1[-1, jh*128+p]
    tvals = np.empty((2, NSTEPS), np.float64)
    tvals[0] = np.arange(NSTEPS) / NSTEPS              # backward t
    tvals[1] = (np.arange(NSTEPS) + 1.0) / NSTEPS      # forward t
    b1r = b1.reshape(NHT, 128).T.astype(np.float64)    # [128, NHT]
    w1t = W1[ZD + CD].reshape(NHT, 128).T.astype(np.float64)
    bia = (b1r[:, :, None, None]
           + w1t[:, :, None, None] * tvals[None, None]).astype(np.float32)

    in_maps = []
    for core in range(NCORES):
        sl = slice(core * BL, (core + 1) * BL)
        in_maps.append({
            "z0": np.ascontiguousarray(z0_t[:, sl]),
            "ctx": np.ascontiguousarray(ctx_t[:, sl]),
            "mut": np.ascontiguousarray(mut_t[:, sl]),
            "eps": np.ascontiguousarray(epr[:, :, sl]),
            "w1z": w1z, "w1c": w1c, "w2": w2s, "idm": idm, "bia": bia,
        })
    return z0, eps, in_maps


def _assemble(results, z0, eps, mu_T, sigma_T, b2, consts):
    """Gather per-core outputs into (log_w, z_chain)."""
    z_chain = np.empty((NSTEPS + 1, B, ZD), np.float32)
    z_chain[0] = z0
    for core, res in enumerate(results):
        sl = slice(core * BL, (core + 1) * BL)
        z_chain[1:, sl, :] = res["zs"].transpose(0, 2, 1)

    s_b = consts["s_b"]
    g = (DT * b2.astype(np.float64)).reshape(NZT, 128).T   # [128, NZT]

    term_d = 0.0
    for res in results:
        st = res["sts"].astype(np.float64)                 # [128,NZT,NBC,N,2]
        mean = st[..., 0]
        var = st[..., 1]
        s = mean + g[:, :, None, None]
        cell = BC * (var + s * s)                          # sum (d+g)^2 per cell
        term_d += (cell.sum(axis=(0, 1, 2)) / s_b**2).sum()
    term_d *= -0.5

    term_f = 0.5 * float(np.square(eps.astype(np.float64)).sum())

    zT = z_chain[-1].astype(np.float64)
    muT = mu_T.astype(np.float64)
    sT = sigma_T.astype(np.float64)
    term_end = float((-0.5 * np.square((zT - muT) / sT) - np.log(sT)
                      + 0.5 * np.square(z0.astype(np.float64))).sum())

    log_w = np.float32((term_d + term_f + term_end) / B)
    return log_w, z_chain


def _run_device(in_maps, trace=False):
    from concourse.bass_utils import run_bass_kernel_spmd

    nc = in_maps.pop("__nc__")
    return run_bass_kernel_spmd(nc, in_maps["maps"], list(range(NCORES)),
                                trace=trace)


def _numpy_fallback(context_embedding, mu_T, sigma_T, W1, b1, W2, b2,
                    sigma_sched, beta_sched):
    """Replicates the reference in numpy fp32 (general sigma_T); slow path."""
    z0, eps = _rng_draws()
    n = NSTEPS
    dtf = np.float32(DT)
    sqdt = np.float32(math.sqrt(DT))
    inv_var = (1.0 / np.square(sigma_T)).astype(np.float32)
    idx_b = (np.arange(n) - 1) % n
    z = z0.copy()
    log_w = np.zeros((B, ZD), np.float32)
    chain = np.empty((n + 1, B, ZD), np.float32)
    chain[0] = z0

    def ctrl(t, zz):
        x = np.concatenate([zz, context_embedding,
                            np.full((B, 1), t, np.float32)], axis=-1)
        return np.tanh(x @ W1 + b1) @ W2 + b2

    def grad(zz, beta):
        return (1.0 - beta) * (-zz) + beta * (-(zz - mu_T) * inv_var)

    def lpdf(x, mu, sig):
        return (-0.5 * np.square((x - mu) / sig) - np.log(sig)
                - np.float32(0.5 * math.log(2 * math.pi)))

    for i in range(n):
        sig_f = np.float32(sigma_sched[i]); bet_f = np.float32(beta_sched[i])
        sig_b = np.float32(sigma_sched[idx_b[i]])
        bet_b = np.float32(beta_sched[idx_b[i]])
        tf = np.float32((i + 1) / n); tb = np.float32(i / n)
        mu_f = z + (sig_f**2 * grad(z, bet_f) + ctrl(tf, z)) * dtf
        s_f = sig_f * sqdt
        z_next = mu_f + s_f * eps[i]
        mu_b = z_next + (sig_b**2 * grad(z_next, bet_b) - ctrl(tb, z_next)) * dtf
        s_b = sig_b * sqdt
        log_w += lpdf(z, mu_b, s_b) - lpdf(z_next, mu_f, s_f)
        z = z_next
        chain[i + 1] = z
    log_w += lpdf(z, mu_T, sigma_T) - (-0.5 * np.square(z0)
                                       - np.float32(0.5 * math.log(2 * math.pi)))
    return np.float32(log_w.mean(axis=0).sum()), chain


def kernel(context_embedding, mu_T, sigma_T, W1, b1, W2, b2,
           sigma_sched, beta_sched, _trace=False):
    context_embedding = np.asarray(context_embedding, np.float32)
    mu_T = np.asarray(mu_T, np.float32)
    sigma_T = np.asarray(sigma_T, np.float32)
    W1 = np.asarray(W1, np.float32)
    b1 = np.asarray(b1, np.float32)
    W2 = np.asarray(W2, np.float32)
    b2 = np.asarray(b2, np.float32)
    sigma_sched = np.asarray(sigma_sched, np.float32)
    beta_sched = np.asarray(beta_sched, np.float32)

    if not np.allclose(sigma_T, 1.0, atol=1e-6):
        return _numpy_fallback(context_embedding, mu_T, sigma_T, W1, b1, W2,
                               b2, sigma_sched, beta_sched)

    consts = _schedule_constants(sigma_sched, beta_sched)
    nc = _get_program(consts)
    z0, eps, in_maps = _host_inputs(context_embedding, mu_T, W1, b1, W2, b2,
                                    consts)

    from concourse.bass_utils import run_bass_kernel_spmd
    kres = run_bass_kernel_spmd(nc, in_maps, list(range(NCORES)), trace=_trace)

    log_w, z_chain = _assemble(kres.results, z0, eps, mu_T, sigma_T, b2, consts)
    if _trace:
        return (log_w, z_chain), kres
    return log_w, z_chain


# revision 5
# speedup vs baseline: 1.0003x; 1.0003x over previous
"""CMCD sampler kernel for Trainium2 (8 NeuronCores, data-parallel over batch).

Problem: nn_CMCD_84877143704251.

reference semantics (B=8192, ZD=256, CD=256, N=32, H=512):
  z0, eps ~ fixed jax RNG (key 42)
  per step i:
    mu_f  = z + (sig_f^2 * grad(z, beta_f) + ctrl(tf, z)) * dt
    z'    = mu_f + sig_f*sqrt(dt)*eps_i
    mu_b  = z' + (sig_b^2 * grad(z', beta_b) - ctrl(tb, z')) * dt
    log_w += lpdf(z, mu_b, s_b) - lpdf(z', mu_f, s_f)
  grad(z, beta) = (1-beta)(-z) + beta*(-(z-mu_T)/sigma_T^2)
  ctrl(t, z) = tanh([z, ctx, t] @ W1 + b1) @ W2 + b2
  outputs: (log_w_scalar, z_chain (N+1, B, ZD))

Device strategy (per core, batch shard BL=1024, feature-major layout
[feature partitions, batch free]):
  - sigma_T == 1 (as produced by setup_inputs) lets grad simplify to
    beta*mu_T - z. Verified at runtime; otherwise fall back to numpy.
  - z' = af_i * z + dt*(tanh(FL + bias_f) @ W2) + e'_i  with
      FL    = z @ W1[:ZD] + ctx @ W1[ZD:ZD+CD]        (PE, ctx part cached)
      bias  = b1 + t * W1[-1]                          (per-partition ACT bias)
      e'_i  = s_f*eps_i + dt*sig_f^2*beta_f*mu_T + dt*b2   (host precomputed)
  - d_i = (z - mu_b) - dt*b2
        = zprev - bh*z' - ch*mu_T + dt*(tanh(FL' + bias_b) @ W2)
    Only per-(partition,step) mean/var of d are shipped out (bn_stats);
    host reconstructs sum((d + dt*b2)^2)/s_b^2.
  - log_w assembled on host: forward quadratic == sum(0.5*eps^2), per-step
    log-sigma constants cancel over the full loop (permutation), endpoint
    terms computed from z_chain on host.
"""

import math
from functools import lru_cache

import numpy as np

B, ZD, CD, NSTEPS, H = 8192, 256, 256, 32, 512
NCORES = 8
BL = B // NCORES           # batch per core
BC = 512                   # batch chunk processed per psum tile
NBC = BL // BC             # 2
NZT = ZD // 128            # zd partition tiles
NCT = CD // 128            # ctx partition tiles
NHT = H // 128             # h partition tiles
DT = 1.0 / NSTEPS

_PROG_CACHE: dict = {}


def _schedule_constants(sigma_sched, beta_sched):
    sf = np.asarray(sigma_sched, np.float64)
    bf = np.asarray(beta_sched, np.float64)
    idx_b = (np.arange(NSTEPS) - 1) % NSTEPS
    sb = sf[idx_b]
    bb = bf[idx_b]
    return {
        "af": 1.0 - DT * sf**2,
        "cf": DT * sf**2 * bf,
        "s_f": sf * math.sqrt(DT),
        "bh": 1.0 - DT * sb**2,
        "ch": DT * sb**2 * bb,
        "s_b": sb * math.sqrt(DT),
    }


def _build_program(af, bh, ch):
    """Emit the per-core Bass program. af/bh/ch are tuples of python floats
    (baked as immediates)."""
    from contextlib import ExitStack

    import concourse.tile as tile
    from concourse import bacc, mybir

    f32 = mybir.dt.float32
    f32r = mybir.dt.float32r
    MULT = mybir.AluOpType.mult
    ADD = mybir.AluOpType.add
    TANH = mybir.ActivationFunctionType.Tanh

    nc = bacc.Bacc("TRN2", target_bir_lowering=False, debug=False,
                   num_devices=NCORES)

    def inp(name, shape, dt=f32):
        return nc.dram_tensor(name, list(shape), dt, kind="ExternalInput").ap()

    def outp(name, shape, dt=f32):
        return nc.dram_tensor(name, list(shape), dt, kind="ExternalOutput").ap()

    z0_d = inp("z0", (ZD, BL))
    ctx_d = inp("ctx", (CD, BL), f32r)
    mut_d = inp("mut", (ZD, BL), f32r)
    eps_d = inp("eps", (NSTEPS, ZD, BL), f32r)
    w1z_d = inp("w1z", (128, NZT, NHT, 128), f32r)
    w1c_d = inp("w1c", (128, NCT, NHT, 128), f32r)
    w2_d = inp("w2", (128, NHT, NZT, 128), f32r)
    idm_d = inp("idm", (128, 128), f32r)
    bia_d = inp("bia", (128, NHT, 2, NSTEPS))
    zs_d = outp("zs", (NSTEPS, ZD, BL))
    sts_d = outp("sts", (128, NZT, NBC, NSTEPS, 2))

    with ExitStack() as ctxs:
        tc = ctxs.enter_context(tile.TileContext(nc))
        const = ctxs.enter_context(tc.tile_pool(name="const", bufs=1))
        zpool = ctxs.enter_context(tc.tile_pool(name="zpool", bufs=3))
        zrpool = ctxs.enter_context(tc.tile_pool(name="zrpool", bufs=3))
        epool = ctxs.enter_context(tc.tile_pool(name="epool", bufs=3))
        hpool = ctxs.enter_context(tc.tile_pool(name="hpool", bufs=4))
        dpool = ctxs.enter_context(tc.tile_pool(name="dpool", bufs=4))
        spool = ctxs.enter_context(tc.tile_pool(name="spool", bufs=4))
        icpool = ctxs.enter_context(tc.tile_pool(name="icpool", bufs=2))
        flp = ctxs.enter_context(tc.tile_pool(name="flp", bufs=1, space="PSUM"))
        psp = ctxs.enter_context(tc.tile_pool(name="psp", bufs=4, space="PSUM"))

        # ---- constants into SBUF ----
        w1z = const.tile([128, NZT, NHT, 128], f32r, name="w1z_sb")
        nc.sync.dma_start(out=w1z[:], in_=w1z_d)
        w1c = const.tile([128, NCT, NHT, 128], f32r, name="w1c_sb")
        nc.sync.dma_start(out=w1c[:], in_=w1c_d)
        w2 = const.tile([128, NHT, NZT, 128], f32r, name="w2_sb")
        nc.sync.dma_start(out=w2[:], in_=w2_d)
        idm = const.tile([128, 128], f32r, name="idm_sb")
        nc.sync.dma_start(out=idm[:], in_=idm_d)
        bia = const.tile([128, NHT, 2, NSTEPS], f32, name="bia_sb")
        nc.sync.dma_start(out=bia[:], in_=bia_d)
        mut = const.tile([128, NZT, BL], f32r, name="mut_sb")
        for j in range(NZT):
            nc.sync.dma_start(out=mut[:, j, :], in_=mut_d[128 * j:128 * (j + 1), :])
        ctxt = const.tile([128, NCT, BL], f32r, name="ctx_sb")
        for j in range(NCT):
            nc.sync.dma_start(out=ctxt[:, j, :], in_=ctx_d[128 * j:128 * (j + 1), :])
        stat = const.tile([128, NZT, NBC, NSTEPS, 2], f32, name="stat_sb")

        # ---- ctx part of first layer (constant over steps) ----
        ctxp = const.tile([128, NHT, BL], f32r, name="ctxp_sb")
        for jh in range(NHT):
            for c in range(NBC):
                sl = slice(c * BC, (c + 1) * BC)
                ps = psp.tile([128, BC], f32, name=f"ctxp_ps_{jh}_{c}", tag="ps")
                for kk in range(NCT):
                    nc.tensor.matmul(ps[:], w1c[:, kk, jh, :], ctxt[:, kk, sl],
                                     start=(kk == 0), stop=(kk == NCT - 1))
                nc.scalar.copy(out=ctxp[:, jh, sl], in_=ps[:])

        # ---- z0 ----
        zcur = zpool.tile([128, NZT, BL], f32, name="z_0", tag="z")
        for j in range(NZT):
            nc.sync.dma_start(out=zcur[:, j, :], in_=z0_d[128 * j:128 * (j + 1), :])
        zrcur = zrpool.tile([128, NZT, BL], f32r, name="zr_0", tag="zr")
        for j in range(NZT):
            nc.vector.tensor_copy(zrcur[:, j, :], zcur[:, j, :])

        def load_eps(k):
            t = epool.tile([128, NZT, BL], f32r, name=f"eps_{k}", tag="eps")
            for j in range(NZT):
                nc.sync.dma_start(out=t[:, j, :],
                                  in_=eps_d[k, 128 * j:128 * (j + 1), :])
            return t

        eps_sb = {0: load_eps(0)}
        zprev = zrprev = None

        # ---- main chain: k indexes the z being fed through the first layer ----
        for k in range(NSTEPS + 1):
            if k + 1 < NSTEPS:
                eps_sb[k + 1] = load_eps(k + 1)
            fwd = k < NSTEPS   # produce z_{k+1}
            bwd = k >= 1       # score step k-1 backward kernel

            if bwd:
                idc = icpool.tile([128, 128], f32r, name=f"idc_{k}", tag="idc")
                nc.vector.tensor_scalar_mul(idc[:], idm[:], float(-ch[k - 1]))

            znext = zrnext = None
            if fwd:
                znext = zpool.tile([128, NZT, BL], f32, name=f"z_{k + 1}", tag="z")
                zrnext = zrpool.tile([128, NZT, BL], f32r, name=f"zr_{k + 1}", tag="zr")

            for c in range(NBC):
                sl = slice(c * BC, (c + 1) * BC)
                # first layer for z_k (shared by bwd step k-1 and fwd step k)
                fl = flp.tile([128, NHT, BC], f32, name=f"fl_{k}_{c}", tag="fl")
                for jh in range(NHT):
                    for kk in range(NZT):
                        nc.tensor.matmul(fl[:, jh, :], w1z[:, kk, jh, :],
                                         zrcur[:, kk, sl],
                                         start=(kk == 0), stop=False)
                    nc.tensor.matmul(fl[:, jh, :], idm[:], ctxp[:, jh, sl],
                                     start=False, stop=True)

                if bwd:
                    hb = hpool.tile([128, NHT, BC], f32r, name=f"hb_{k}_{c}", tag="h")
                    for jh in range(NHT):
                        nc.scalar.activation(hb[:, jh, :], fl[:, jh, :], TANH,
                                             bias=bia[:, jh, 0, k - 1:k], scale=1.0)
                hf = None
                if fwd:
                    hf = hpool.tile([128, NHT, BC], f32r, name=f"hf_{k}_{c}", tag="h")
                    for jh in range(NHT):
                        nc.scalar.activation(hf[:, jh, :], fl[:, jh, :], TANH,
                                             bias=bia[:, jh, 1, k:k + 1], scale=1.0)

                if bwd:
                    # d = zprev - bh*z_k - ch*mu_T + dt*(hb @ W2)
                    for j in range(NZT):
                        dps = psp.tile([128, BC], f32, name=f"dps_{k}_{c}_{j}",
                                       tag="ps")
                        for kk in range(NHT):
                            nc.tensor.matmul(dps[:], w2[:, kk, j, :],
                                             hb[:, kk, :],
                                             start=(kk == 0), stop=False)
                        nc.tensor.matmul(dps[:], idm[:], zrprev[:, j, sl],
                                         start=False, stop=False)
                        nc.tensor.matmul(dps[:], idc[:], mut[:, j, sl],
                                         start=False, stop=True)
                        dsb = dpool.tile([128, BC], f32, name=f"d_{k}_{c}_{j}",
                                         tag="d")
                        nc.vector.scalar_tensor_tensor(
                            dsb[:], zcur[:, j, sl], float(-bh[k - 1]), dps[:],
                            MULT, ADD)
                        st6 = spool.tile([128, 6], f32, name=f"st6_{k}_{c}_{j}",
                                         tag="st6")
                        nc.vector.bn_stats(st6[:], dsb[:])
                        nc.vector.bn_aggr(stat[:, j, c, k - 1, :], st6[:])

                if fwd:
                    # z_{k+1} = af*z_k + dt*(hf @ W2) + e'_k
                    for j in range(NZT):
                        zps = psp.tile([128, BC], f32, name=f"zps_{k}_{c}_{j}",
                                       tag="ps")
                        for kk in range(NHT):
                            nc.tensor.matmul(zps[:], w2[:, kk, j, :],
                                             hf[:, kk, :],
                                             start=(kk == 0), stop=False)
                        nc.tensor.matmul(zps[:], idm[:], eps_sb[k][:, j, sl],
                                         start=False, stop=True)
                        nc.vector.scalar_tensor_tensor(
                            znext[:, j, sl], zcur[:, j, sl], float(af[k]), zps[:],
                            MULT, ADD)
                        nc.vector.tensor_copy(zrnext[:, j, sl], znext[:, j, sl])
                        nc.sync.dma_start(out=zs_d[k, 128 * j:128 * (j + 1), sl],
                                          in_=znext[:, j, sl])

            if fwd:
                zprev = zcur
                zcur = znext
                zrprev = zrcur
                zrcur = zrnext
            if k - 1 in eps_sb:
                del eps_sb[k - 1]

        nc.sync.dma_start(out=sts_d, in_=stat[:])

    nc.compile()
    return nc


def _get_program(consts):
    key = (tuple(consts["af"]), tuple(consts["bh"]), tuple(consts["ch"]))
    if key not in _PROG_CACHE:
        _PROG_CACHE.clear()
        _PROG_CACHE[key] = _build_program(consts["af"], consts["bh"], consts["ch"])
    return _PROG_CACHE[key]


_RNG_SCRIPT = """
import sys
import jax
import numpy as np
key = jax.random.key(42)
k0, k1 = jax.random.split(key)
z0 = np.asarray(jax.random.normal(k0, ({B}, {ZD}), np.float32))
eps = np.asarray(jax.random.normal(k1, ({N}, {B}, {ZD}), np.float32))
np.save(sys.argv[1], z0)
np.save(sys.argv[2], eps)
"""


@lru_cache(maxsize=1)
def _rng_draws():
    """Draw z0/eps exactly as the reference does. The reference runs under
    JAX_PLATFORMS=cpu and this container pins jax_default_prng_impl=rbg,
    whose bitstream is backend-dependent — so the draw must be lowered for
    the CPU backend. Done in a subprocess to be independent of this
    process's jax platform state."""
    import os
    import subprocess
    import sys
    import tempfile

    with tempfile.TemporaryDirectory() as td:
        pz = os.path.join(td, "z0.npy")
        pe = os.path.join(td, "eps.npy")
        env = dict(os.environ, JAX_PLATFORMS="cpu")
        script = _RNG_SCRIPT.format(B=B, ZD=ZD, N=NSTEPS)
        subprocess.run([sys.executable, "-c", script, pz, pe], env=env,
                       check=True, capture_output=True)
        z0 = np.load(pz)
        eps = np.load(pe)
    return z0, eps


def _host_inputs(context_embedding, mu_T, W1, b1, W2, b2, consts):
    z0, eps = _rng_draws()
    s_f = consts["s_f"].astype(np.float32)
    cf = consts["cf"].astype(np.float32)

    mut_t = np.ascontiguousarray(mu_T.T)                    # (ZD, B)
    ctx_t = np.ascontiguousarray(context_embedding.T)       # (CD, B)
    z0_t = np.ascontiguousarray(z0.T)                       # (ZD, B)
    # e' = s_f*eps + dt*sig_f^2*beta_f*mu_T + dt*b2  (feature-major)
    eps_t = np.ascontiguousarray(eps.transpose(0, 2, 1))    # (N, ZD, B)
    epr = (s_f[:, None, None] * eps_t
           + cf[:, None, None] * mut_t[None]
           + (DT * b2).astype(np.float32)[None, :, None])

    w1z = np.ascontiguousarray(
        W1[:ZD].reshape(NZT, 128, NHT, 128).transpose(1, 0, 2, 3))
    w1c = np.ascontiguousarray(
        W1[ZD:ZD + CD].reshape(NCT, 128, NHT, 128).transpose(1, 0, 2, 3))
    w2s = np.ascontiguousarray(
        (DT * W2.astype(np.float64)).astype(np.float32)
        .reshape(NHT, 128, NZT, 128).transpose(1, 0, 2, 3))
    idm = np.eye(128, dtype=np.float32)

    # bias[p, jh, kind, i] = b1[jh*128+p] + t * W1[-1, jh*128+p]
    tvals = np.empty((2, NSTEPS), np.float64)
    tvals[0] = np.arange(NSTEPS) / NSTEPS              # backward t
    tvals[1] = (np.arange(NSTEPS) + 1.0) / NSTEPS      # forward t
    b1r = b1.reshape(NHT, 128).T.astype(np.float64)    # [128, NHT]
    w1t = W1[ZD + CD].reshape(NHT, 128).T.astype(np.float64)
    bia = (b1r[:, :, None, None]
           + w1t[:, :, None, None] * tvals[None, None]).astype(np.float32)

    in_maps = []
    for core in range(NCORES):
        sl = slice(core * BL, (core + 1) * BL)
        in_maps.append({
            "z0": np.ascontiguousarray(z0_t[:, sl]),
            "ctx": np.ascontiguousarray(ctx_t[:, sl]),
            "mut": np.ascontiguousarray(mut_t[:, sl]),
            "eps": np.ascontiguousarray(epr[:, :, sl]),
            "w1z": w1z, "w1c": w1c, "w2": w2s, "idm": idm, "bia": bia,
        })
    return z0, eps, in_maps


def _assemble(results, z0, eps, mu_T, sigma_T, b2, consts):
    """Gather per-core outputs into (log_w, z_chain)."""
    z_chain = np.empty((NSTEPS + 1, B, ZD), np.float32)
    z_chain[0] = z0
    for core, res in enumerate(results):
        sl = slice(core * BL, (core + 1) * BL)
        z_chain[1:, sl, :] = res["zs"].transpose(0, 2, 1)

    s_b = consts["s_b"]
    g = (DT * b2.astype(np.float64)).reshape(NZT, 128).T   # [128, NZT]

    term_d = 0.0
    for res in results:
        st = res["sts"].astype(np.float64)                 # [128,NZT,NBC,N,2]
        mean = st[..., 0]
        var = st[..., 1]
        s = mean + g[:, :, None, None]
        cell = BC * (var + s * s)                          # sum (d+g)^2 per cell
        term_d += (cell.sum(axis=(0, 1, 2)) / s_b**2).sum()
    term_d *= -0.5

    term_f = 0.5 * float(np.square(eps.astype(np.float64)).sum())

    zT = z_chain[-1].astype(np.float64)
    muT = mu_T.astype(np.float64)
    sT = sigma_T.astype(np.float64)
    term_end = float((-0.5 * np.square((zT - muT) / sT) - np.log(sT)
                      + 0.5 * np.square(z0.astype(np.float64))).sum())

    log_w = np.float32((term_d + term_f + term_end) / B)
    return log_w, z_chain


def _run_device(in_maps, trace=False):
    from concourse.bass_utils import run_bass_kernel_spmd

    nc = in_maps.pop("__nc__")
    return run_bass_kernel_spmd(nc, in_maps["maps"], list(range(NCORES)),
                                trace=trace)


def _numpy_fallback(context_embedding, mu_T, sigma_T, W1, b1, W2, b2,
                    sigma_sched, beta_sched):
    """Replicates the reference in numpy fp32 (general sigma_T); slow path."""
    z0, eps = _rng_draws()
    n = NSTEPS
    dtf = np.float32(DT)
    sqdt = np.float32(math.sqrt(DT))
    inv_var = (1.0 / np.square(sigma_T)).astype(np.float32)
    idx_b = (np.arange(n) - 1) % n
    z = z0.copy()
    log_w = np.zeros((B, ZD), np.float32)
    chain = np.empty((n + 1, B, ZD), np.float32)
    chain[0] = z0

    def ctrl(t, zz):
        x = np.concatenate([zz, context_embedding,
                            np.full((B, 1), t, np.float32)], axis=-1)
        return np.tanh(x @ W1 + b1) @ W2 + b2

    def grad(zz, beta):
        return (1.0 - beta) * (-zz) + beta * (-(zz - mu_T) * inv_var)

    def lpdf(x, mu, sig):
        return (-0.5 * np.square((x - mu) / sig) - np.log(sig)
                - np.float32(0.5 * math.log(2 * math.pi)))

    for i in range(n):
        sig_f = np.float32(sigma_sched[i]); bet_f = np.float32(beta_sched[i])
        sig_b = np.float32(sigma_sched[idx_b[i]])
        bet_b = np.float32(beta_sched[idx_b[i]])
        tf = np.float32((i + 1) / n); tb = np.float32(i / n)
        mu_f = z + (sig_f**2 * grad(z, bet_f) + ctrl(tf, z)) * dtf
        s_f = sig_f * sqdt
        z_next = mu_f + s_f * eps[i]
        mu_b = z_next + (sig_b**2 * grad(z_next, bet_b) - ctrl(tb, z_next)) * dtf
        s_b = sig_b * sqdt
        log_w += lpdf(z, mu_b, s_b) - lpdf(z_next, mu_f, s_f)
        z = z_next
        chain[i + 1] = z
    log_w += lpdf(z, mu_T, sigma_T) - (-0.5 * np.square(z0)
                                       - np.float32(0.5 * math.log(2 * math.pi)))
    return np.float32(log_w.mean(axis=0).sum()), chain


def kernel(context_embedding, mu_T, sigma_T, W1, b1, W2, b2,
           sigma_sched, beta_sched, _trace=False):
    context_embedding = np.asarray(context_embedding, np.float32)
    mu_T = np.asarray(mu_T, np.float32)
    sigma_T = np.asarray(sigma_T, np.float32)
    W1 = np.asarray(W1, np.float32)
    b1 = np.asarray(b1, np.float32)
    W2 = np.asarray(W2, np.float32)
    b2 = np.asarray(b2, np.float32)
    sigma_sched = np.asarray(sigma_sched, np.float32)
    beta_sched = np.asarray(beta_sched, np.float32)

    if not np.allclose(sigma_T, 1.0, atol=1e-6):
        return _numpy_fallback(context_embedding, mu_T, sigma_T, W1, b1, W2,
                               b2, sigma_sched, beta_sched)

    consts = _schedule_constants(sigma_sched, beta_sched)
    nc = _get_program(consts)
    z0, eps, in_maps = _host_inputs(context_embedding, mu_T, W1, b1, W2, b2,
                                    consts)

    from concourse.bass_utils import run_bass_kernel_spmd
    kres = run_bass_kernel_spmd(nc, in_maps, list(range(NCORES)), trace=_trace)

    log_w, z_chain = _assemble(kres.results, z0, eps, mu_T, sigma_T, b2, consts)
    if _trace:
        return (log_w, z_chain), kres
    return log_w, z_chain
